# revision 39
# baseline (speedup 1.0000x reference)
"""DND kNN kernel v4 — dma_gather over 4 SWDGE queues.

Sharding: the fused table (bf16 keys + f32-value bits, padded to 256 B rows)
is split row-wise into 8 shards of 250 000 rows, one per core; each shard is
further split into 8 windows of 31 250 rows so local row ids fit int16 (the
dma_gather index dtype).  Each (query, neighbor) pair is routed on the host to
the core owning its table row, sorted by (window, local row), and padded per
window to a static slot count C_cap (multiple of 128).  Host-side prep also
builds, per core, the int16 index tiles (wrapped in 16 partitions, replicated
for the 8 Q7 cores) and a slot-aligned copy of the query vectors (kr).

Device, per core: each window's rows are fetched by 1024-idx dma_gather calls
rotated across all 4 SWDGE queues (num_swdge_queues=4).  A single gather's
drain is latency-bound — its descriptors stripe over 16 DMA engines that each
process ~64 serial ~140 ns random 256 B HBM reads — so queue rotation is the
big lever: 4 in-flight ring drains ≈ 2.8x over one queue (290 us -> 103 us).
Slots are additionally permuted so each DMA engine's serial descriptor stream
walks ascending table rows, and the last window is issued as 512-idx chunks
so the end-of-kernel drains overlap across queues.  A custom DVE op computes
a running sum of (row - query)^2 per partition; per-row squared distances
come out as differences of the cumsum at 64-element boundaries; w =
1/(d2+delta) and w*v land in a [128, S, 2] tile that is DMA'd out.  Host
epilogue: segmented sum of (w, w*v) by query id and the final division — the
unshard step for this table-parallel sharding.

Remaining time budget (measured): ~17 us prologue gated by the Q7 'mlp'
library image DMA (fixed), ~71 us of gather drain at the 64-outstanding
latency ceiling, ~12 us drain tail + end barrier.
"""

import sys

import numpy as np

for _p in ("/opt/trn_rl_repo",):
    if _p not in sys.path:
        sys.path.insert(0, _p)

CAPACITY = 2_000_000
DIM = 64
K = 50
BATCH = 4096
DELTA = 1e-3
N_CORES = 8
P = 128
ROW = 128  # bf16 elems per fused row = 256 B
WPC = 8  # windows per core
N_WINDOWS = N_CORES * WPC  # 64
WROWS = CAPACITY // N_WINDOWS  # 31250 rows per window, int16-addressable

_BUILD_CACHE = {}


def _register_cumsqdiff():
    from concourse import dve_ops
    from concourse.dve_spec import AluOp, Spec, Src0, Src1, _has_src1, lower, scan, sq
    from concourse.dve_uop import DveOpSpec

    for op in dve_ops.OPS:
        if op.name == "CUMSQDIFF_ANT":
            return op

    def _ref(in0, in1, c0, c1, c2):
        a = in0.astype(np.float32).reshape(in0.shape[0], -1)
        b = in1.astype(np.float32).reshape(in1.shape[0], -1)
        d = a - b
        s = np.cumsum(d * d, axis=1, dtype=np.float32)
        return s.reshape(in0.shape)

    spec = Spec(body=scan(AluOp.ADD, sq(Src0 - Src1)), reference=_ref)
    name = "CUMSQDIFF_ANT"
    opcode = dve_ops._CUSTOM_DVE_ROW_BASE + len(dve_ops.OPS)
    shas = {}
    for ver in ("v3", "v4"):
        s = DveOpSpec(
            name=name, opcode=opcode, uops=lower(spec, ver=ver), rd1_en=_has_src1(spec)
        )
        shas[ver] = s.sha(ver)
    op = dve_ops.DveOp(name, spec, subdim=False, uops_sha=shas)
    dve_ops.OPS.append(op)
    dve_ops.CUSTOM_DVE_SPECS[name] = spec
    dve_ops._SUB_OPCODE_FOR_NAME[name] = opcode
    return op


def _build(c_cap, wrows=WROWS, wpc=WPC, num_devices=N_CORES):
    """Per-core program. c_cap = slots per window (multiple of 128)."""
    key = (c_cap, wrows, wpc, num_devices)
    if key in _BUILD_CACHE:
        return _BUILD_CACHE[key]

    from contextlib import ExitStack

    import concourse.bacc as bacc
    import concourse.tile as tile
    from concourse import mybir

    f32 = mybir.dt.float32
    bf16 = mybir.dt.bfloat16
    i16 = mybir.dt.int16
    cumsqdiff = _register_cumsqdiff()

    cc = c_cap // P  # gather out columns per window
    S = wpc * cc  # total slot columns per core

    nc = bacc.Bacc(
        "TRN2",
        target_bir_lowering=False,
        debug=False,
        num_devices=num_devices,
        num_swdge_queues=4,
    )
    SUBMAX = 1024  # dma_gather num_idxs hard cap (measured; >1024 faults)
    subs = []
    r = c_cap
    while r > 0:
        s = min(r, SUBMAX)
        subs.append(s)
        r -= s
    ft = nc.dram_tensor("ft_shard", [wpc * wrows, ROW], bf16, kind="ExternalInput")
    kr = nc.dram_tensor("kr", [P, S * DIM], bf16, kind="ExternalInput")
    idxs = nc.dram_tensor("idxs", [P, wpc * c_cap // 16], i16, kind="ExternalInput")
    w2o = nc.dram_tensor("w2", [P, S * 2], f32, kind="ExternalOutput")

    with tile.TileContext(nc) as tc, ExitStack() as ctx:
        io_pool = ctx.enter_context(tc.tile_pool(name="io", bufs=1))
        g_pool = ctx.enter_context(tc.tile_pool(name="g", bufs=2))
        wk_pool = ctx.enter_context(tc.tile_pool(name="wk", bufs=2))

        # chunked loads: smaller DMAs interleave better on the engine queues
        # than one monolithic transfer (measured)
        idx_t = io_pool.tile([P, wpc * c_cap // 16], i16, tag="idx")
        for w in range(wpc):
            nc.sync.dma_start(
                out=idx_t[:, w * (c_cap // 16) : (w + 1) * (c_cap // 16)],
                in_=idxs[:, w * (c_cap // 16) : (w + 1) * (c_cap // 16)],
            )
        w2 = io_pool.tile([P, S * 2], f32, tag="w2")
        w23 = w2[:].rearrange("p (s two) -> p s two", two=2)

        kr_all = io_pool.tile([P, S * DIM], bf16, tag="kr_all")
        for w in range(wpc):
            nc.scalar.dma_start(
                out=kr_all[:, w * cc * DIM : (w + 1) * cc * DIM],
                in_=kr[:, w * cc * DIM : (w + 1) * cc * DIM],
            )
        kr3 = kr_all[:].rearrange("p (s d) -> p s d", d=DIM)

        # last window in 512-chunks, ending in two 128-chunks: spreads the
        # final drains over the queues AND makes the very last ring drain
        # (the tail critical path) ~4x shorter
        subs_last = []
        r = c_cap
        while r > 256:
            t = min(r - 256, 512)
            subs_last.append(t)
            r -= t
        while r > 0:
            t = min(r, 128)
            subs_last.append(t)
            r -= t

        gq = 0  # rotate SWDGE queues so DMA ring drains overlap
        for w in range(wpc):
            G = g_pool.tile([P, cc * ROW], bf16, tag="G")
            G3w = G[:].rearrange("p (c e) -> p c e", e=ROW)
            off = 0
            gq = 0
            for s in subs_last if w == wpc - 1 else subs:
                nc.gpsimd.dma_gather(
                    G3w[:, off // P : (off + s) // P, :],
                    ft[w * wrows : (w + 1) * wrows, :],
                    idx_t[
                        :,
                        (w * c_cap + off) // 16 : (w * c_cap + off + s) // 16,
                    ],
                    num_idxs=s,
                    num_idxs_reg=s,
                    elem_size=ROW,
                    queue_num=gq % 4,
                )
                gq += 1
                off += s
            G3 = G[:].rearrange("p (c e) -> p c e", e=ROW)

            def piece(lo, hi, w=w, G3=G3):
                n = hi - lo
                cum = wk_pool.tile([P, (cc + 1) * DIM], f32, tag="cum")
                nc.vector.memset(cum[:, DIM - 1 : DIM], 0)
                nc.vector._custom_dve(
                    cumsqdiff,
                    out=cum[:, DIM : (n + 1) * DIM],
                    in0=G3[:, lo:hi, 0:DIM],
                    in1=kr3[:, w * cc + lo : w * cc + hi, :],
                )
                cum3 = cum[:].rearrange("p (c d) -> p c d", d=DIM)
                dpd = wk_pool.tile([P, cc], f32, tag="dpd")
                nc.vector.scalar_tensor_tensor(
                    out=dpd[:, 0:n].unsqueeze(-1),
                    in0=cum3[:, 1 : n + 1, DIM - 1 : DIM],
                    scalar=DELTA,
                    in1=cum3[:, 0:n, DIM - 1 : DIM],
                    op0=mybir.AluOpType.add,
                    op1=mybir.AluOpType.subtract,
                )
                wslice = w23[:, w * cc + lo : w * cc + hi, 0:1]
                nc.vector.reciprocal(wslice, dpd[:, 0:n].unsqueeze(-1))
                vv = G3[:, lo:hi, DIM : DIM + 2].bitcast(f32)
                nc.vector.tensor_tensor(
                    out=w23[:, w * cc + lo : w * cc + hi, 1:2],
                    in0=wslice,
                    in1=vv,
                    op=mybir.AluOpType.mult,
                )
                nc.scalar.dma_start(
                    out=w2o[:, (w * cc + lo) * 2 : (w * cc + hi) * 2],
                    in_=w2[:, (w * cc + lo) * 2 : (w * cc + hi) * 2],
                )

            if w == wpc - 1:
                # per-sub pieces so the final compute tail is one 512-chunk
                o = 0
                for s in subs_last:
                    piece(o // P, (o + s) // P)
                    o += s
            else:
                piece(0, cc)

    nc.compile()
    _BUILD_CACHE[key] = nc
    return nc


def _bf16(x):
    import ml_dtypes

    return np.asarray(x, dtype=np.float32).astype(ml_dtypes.bfloat16)


def _make_fused_table(keys_table, values_table, cap):
    import ml_dtypes

    fused = np.zeros((cap, ROW), dtype=ml_dtypes.bfloat16)
    fused[:, :DIM] = _bf16(keys_table)
    v32 = np.ascontiguousarray(np.asarray(values_table, dtype=np.float32).reshape(-1))
    fused_u16 = fused.view(np.uint16)
    fused_u16[:, DIM : DIM + 2] = v32.view(np.uint16).reshape(cap, 2)
    return fused


def _host_prep(key, indices, keys_table, values_table, wrows=WROWS, wpc=WPC,
               n_cores=N_CORES):
    """Returns (fused, per-core in_maps, epilogue info, c_cap)."""
    cap = keys_table.shape[0]
    batch, k = indices.shape
    n_windows = n_cores * wpc
    fused = _make_fused_table(keys_table, values_table, cap)
    key_bf = _bf16(key)  # [batch, DIM]

    i_all = np.asarray(indices).reshape(-1).astype(np.int64)  # [batch*k]
    b_all = np.repeat(np.arange(batch, dtype=np.int64), k)
    w_id = i_all // wrows
    local = (i_all - w_id * wrows).astype(np.int16)

    # sort by (window, local row): ascending DRAM addresses within each
    # window give the gather's DMA descriptors page/bank locality
    order = np.lexsort((local, w_id))
    w_sorted = w_id[order]
    local_sorted = local[order]
    b_sorted = b_all[order]

    counts = np.bincount(w_id, minlength=n_windows)
    c_cap = max(P, int(-(-counts.max() // P)) * P)

    # Engine-contiguous slot permutation: the gather ucode routes the
    # descriptor for slot position 128*k + first[l] + offs[g] to DMA engine
    # l, which processes its descriptors serially.  Placing CONSECUTIVE
    # sorted rows on one engine's stream turns its serial random reads into
    # an ascending walk (DRAM page/row-buffer locality).
    _first = np.array(
        [0, 64, 4, 68, 8, 72, 12, 76, 16, 80, 20, 84, 24, 88, 28, 92]
    )
    _offs = np.array([0, 1, 2, 3, 32, 33, 34, 35])

    def _eperm(s):
        pp = np.empty(s, dtype=np.int64)
        spe = s // 16
        for el in range(16):
            for k in range(s // 128):
                for g in range(8):
                    pp[128 * k + _first[el] + _offs[g]] = el * spe + k * 8 + g
        return pp

    def _window_perm(subs):
        pp = np.empty(c_cap, dtype=np.int64)
        off = 0
        for s in subs:
            pp[off : off + s] = off + _eperm(s)
            off += s
        return pp

    subs_main = []
    r = c_cap
    while r > 0:
        t = min(r, 1024)
        subs_main.append(t)
        r -= t
    # must mirror the device's subs_last construction exactly
    subs_last = []
    r = c_cap
    while r > 256:
        t = min(r - 256, 512)
        subs_last.append(t)
        r -= t
    while r > 0:
        t = min(r, 128)
        subs_last.append(t)
        r -= t
    pos_to_pair = _window_perm(subs_main)
    pos_to_pair_last = _window_perm(subs_last)

    # [n_windows, c_cap] slot arrays, dup-padded with row 0 (all-valid lists:
    # the -1-skip path is flaky on HW; pads are gathered and dropped on host)
    local_pad = np.zeros((n_windows, c_cap), dtype=np.int16)
    b_pad = np.zeros((n_windows, c_cap), dtype=np.int64)
    valid = np.zeros((n_windows, c_cap), dtype=bool)
    starts = np.concatenate([[0], np.cumsum(counts)])
    for w in range(n_windows):
        n = counts[w]
        sl = slice(starts[w], starts[w] + n)
        loc_seq = np.zeros(c_cap, dtype=np.int16)
        loc_seq[:n] = local_sorted[sl]
        b_seq = np.zeros(c_cap, dtype=np.int64)
        b_seq[:n] = b_sorted[sl]
        v_seq = np.zeros(c_cap, dtype=bool)
        v_seq[:n] = True
        pp = pos_to_pair_last if (w % wpc) == wpc - 1 else pos_to_pair
        local_pad[w] = loc_seq[pp]
        b_pad[w] = b_seq[pp]
        valid[w] = v_seq[pp]

    cc = c_cap // P
    in_maps = []
    for c in range(n_cores):
        ws = slice(c * wpc, (c + 1) * wpc)
        # int16 idx tile: index j of window w at [16*rep + j%16, w*c_cap//16 + j//16]
        lp = local_pad[ws]  # [wpc, c_cap]
        arr = lp.reshape(wpc, c_cap // 16, 16).transpose(2, 0, 1).reshape(
            16, wpc * (c_cap // 16)
        )
        idx_tile = np.tile(arr, (8, 1))  # replicate to 128 partitions
        # kr: slot j of window w lands at [j%128, w*cc + j//128]
        bp = b_pad[ws].reshape(wpc, cc, P)  # [w, col, p]
        kr_arr = key_bf[bp]  # [w, col, p, DIM]
        kr_tile = np.ascontiguousarray(
            kr_arr.transpose(2, 0, 1, 3).reshape(P, wpc * cc * DIM)
        )
        in_maps.append(
            {
                "ft_shard": fused[c * wpc * wrows : (c + 1) * wpc * wrows],
                "kr": kr_tile,
                "idxs": np.ascontiguousarray(idx_tile),
            }
        )
    epi = (b_pad, valid, batch)
    return in_maps, epi, c_cap


def _epilogue(results, epi, wpc=WPC, n_cores=N_CORES):
    b_pad, valid, batch = epi
    den = np.zeros(batch, dtype=np.float64)
    num = np.zeros(batch, dtype=np.float64)
    c_cap = b_pad.shape[1]
    cc = c_cap // P
    for c in range(n_cores):
        w2 = results[c]["w2"].reshape(P, wpc * cc, 2)  # [p, col, 2]
        # slot j of window w is at [j%128, w*cc + j//128]
        w2_slots = w2.reshape(P, wpc, cc, 2).transpose(1, 2, 0, 3).reshape(
            wpc, c_cap, 2
        )
        v = valid[c * wpc : (c + 1) * wpc]
        b = b_pad[c * wpc : (c + 1) * wpc]
        den += np.bincount(b[v], weights=w2_slots[..., 0][v], minlength=batch)
        num += np.bincount(b[v], weights=w2_slots[..., 1][v], minlength=batch)
    return (num / den).astype(np.float32)


LAST_RESULTS = None


def kernel(key, indices, keys_table, values_table):
    global LAST_RESULTS
    from concourse.bass_utils import run_bass_kernel_spmd

    in_maps, epi, c_cap = _host_prep(key, indices, keys_table, values_table)
    nc = _build(c_cap)
    res = run_bass_kernel_spmd(nc, in_maps, core_ids=list(range(N_CORES)))
    LAST_RESULTS = res
    return _epilogue(res.results, epi)



# revision 40
# speedup vs baseline: 1.0142x; 1.0142x over previous
"""DND kNN kernel v4 — dma_gather over 4 SWDGE queues.

Sharding: the fused table (bf16 keys + f32-value bits, padded to 256 B rows)
is split row-wise into 8 shards of 250 000 rows, one per core; each shard is
further split into 8 windows of 31 250 rows so local row ids fit int16 (the
dma_gather index dtype).  Each (query, neighbor) pair is routed on the host to
the core owning its table row, sorted by (window, local row), and padded per
window to a static slot count C_cap (multiple of 128).  Host-side prep also
builds, per core, the int16 index tiles (wrapped in 16 partitions, replicated
for the 8 Q7 cores) and a slot-aligned copy of the query vectors (kr).

Device, per core: each window's rows are fetched by 1024-idx dma_gather calls
rotated across all 4 SWDGE queues (num_swdge_queues=4).  A single gather's
drain is latency-bound — its descriptors stripe over 16 DMA engines that each
process ~64 serial ~140 ns random 256 B HBM reads — so queue rotation is the
big lever: 4 in-flight ring drains ≈ 2.8x over one queue (290 us -> 103 us).
Slots are additionally permuted so each DMA engine's serial descriptor stream
walks ascending table rows, and the last window is issued as 512-idx chunks
so the end-of-kernel drains overlap across queues.  A custom DVE op computes
a running sum of (row - query)^2 per partition; per-row squared distances
come out as differences of the cumsum at 64-element boundaries; w =
1/(d2+delta) and w*v land in a [128, S, 2] tile that is DMA'd out.  Host
epilogue: segmented sum of (w, w*v) by query id and the final division — the
unshard step for this table-parallel sharding.

Remaining time budget (measured): ~17 us prologue gated by the Q7 'mlp'
library image DMA (fixed), ~71 us of gather drain at the 64-outstanding
latency ceiling, ~12 us drain tail + end barrier.
"""

import sys

import numpy as np

for _p in ("/opt/trn_rl_repo",):
    if _p not in sys.path:
        sys.path.insert(0, _p)

CAPACITY = 2_000_000
DIM = 64
K = 50
BATCH = 4096
DELTA = 1e-3
N_CORES = 8
P = 128
ROW = 128  # bf16 elems per fused row = 256 B
WPC = 8  # windows per core
N_WINDOWS = N_CORES * WPC  # 64
WROWS = CAPACITY // N_WINDOWS  # 31250 rows per window, int16-addressable

_BUILD_CACHE = {}


def _register_cumsqdiff():
    from concourse import dve_ops
    from concourse.dve_spec import AluOp, Spec, Src0, Src1, _has_src1, lower, scan, sq
    from concourse.dve_uop import DveOpSpec

    for op in dve_ops.OPS:
        if op.name == "CUMSQDIFF_ANT":
            return op

    def _ref(in0, in1, c0, c1, c2):
        a = in0.astype(np.float32).reshape(in0.shape[0], -1)
        b = in1.astype(np.float32).reshape(in1.shape[0], -1)
        d = a - b
        s = np.cumsum(d * d, axis=1, dtype=np.float32)
        return s.reshape(in0.shape)

    spec = Spec(body=scan(AluOp.ADD, sq(Src0 - Src1)), reference=_ref)
    name = "CUMSQDIFF_ANT"
    opcode = dve_ops._CUSTOM_DVE_ROW_BASE + len(dve_ops.OPS)
    shas = {}
    for ver in ("v3", "v4"):
        s = DveOpSpec(
            name=name, opcode=opcode, uops=lower(spec, ver=ver), rd1_en=_has_src1(spec)
        )
        shas[ver] = s.sha(ver)
    op = dve_ops.DveOp(name, spec, subdim=False, uops_sha=shas)
    dve_ops.OPS.append(op)
    dve_ops.CUSTOM_DVE_SPECS[name] = spec
    dve_ops._SUB_OPCODE_FOR_NAME[name] = opcode
    return op


def _build(c_cap, wrows=WROWS, wpc=WPC, num_devices=N_CORES):
    """Per-core program. c_cap = slots per window (multiple of 128)."""
    key = (c_cap, wrows, wpc, num_devices)
    if key in _BUILD_CACHE:
        return _BUILD_CACHE[key]

    from contextlib import ExitStack

    import concourse.bacc as bacc
    import concourse.tile as tile
    from concourse import mybir

    f32 = mybir.dt.float32
    bf16 = mybir.dt.bfloat16
    i16 = mybir.dt.int16
    cumsqdiff = _register_cumsqdiff()

    cc = c_cap // P  # gather out columns per window
    S = wpc * cc  # total slot columns per core

    nc = bacc.Bacc(
        "TRN2",
        target_bir_lowering=False,
        debug=False,
        num_devices=num_devices,
        num_swdge_queues=4,
    )
    SUBMAX = 1024  # dma_gather num_idxs hard cap (measured; >1024 faults)
    subs = []
    r = c_cap
    while r > 0:
        s = min(r, SUBMAX)
        subs.append(s)
        r -= s
    ft = nc.dram_tensor("ft_shard", [wpc * wrows, ROW], bf16, kind="ExternalInput")
    kr = nc.dram_tensor("kr", [P, S * DIM], bf16, kind="ExternalInput")
    idxs = nc.dram_tensor("idxs", [P, wpc * c_cap // 16], i16, kind="ExternalInput")
    w2o = nc.dram_tensor("w2", [P, S * 2], f32, kind="ExternalOutput")

    with tile.TileContext(nc) as tc, ExitStack() as ctx:
        io_pool = ctx.enter_context(tc.tile_pool(name="io", bufs=1))
        g_pool = ctx.enter_context(tc.tile_pool(name="g", bufs=2))
        wk_pool = ctx.enter_context(tc.tile_pool(name="wk", bufs=2))

        # chunked loads: smaller DMAs interleave better on the engine queues
        # than one monolithic transfer (measured)
        idx_t = io_pool.tile([P, wpc * c_cap // 16], i16, tag="idx")
        for w in range(wpc):
            nc.sync.dma_start(
                out=idx_t[:, w * (c_cap // 16) : (w + 1) * (c_cap // 16)],
                in_=idxs[:, w * (c_cap // 16) : (w + 1) * (c_cap // 16)],
            )
        w2 = io_pool.tile([P, S * 2], f32, tag="w2")
        w23 = w2[:].rearrange("p (s two) -> p s two", two=2)

        # per-window kr tiles; only windows 0-1 load up front (the prologue
        # is HBM-bound on the Q7 library image DMA), the rest are emitted
        # after earlier windows' out-DMAs in the scalar engine stream
        kr_t = []
        for w in range(wpc):
            krw = io_pool.tile([P, cc * DIM], bf16, tag=f"kr{w}", name=f"krt{w}")
            kr_t.append(krw)
        kr3_t = [t[:].rearrange("p (s d) -> p s d", d=DIM) for t in kr_t]

        def load_kr(w):
            nc.scalar.dma_start(
                out=kr_t[w][:],
                in_=kr[:, w * cc * DIM : (w + 1) * cc * DIM],
            )

        load_kr(0)
        load_kr(1)

        # last window gathers in 512-chunks: spreads the final drains over
        # all 4 queues so the end-of-kernel DRAIN tail is shorter
        subs_last = []
        r = c_cap
        while r > 0:
            t = min(r, 512)
            subs_last.append(t)
            r -= t

        gq = 0  # rotate SWDGE queues so DMA ring drains overlap
        for w in range(wpc):
            G = g_pool.tile([P, cc * ROW], bf16, tag="G")
            G3w = G[:].rearrange("p (c e) -> p c e", e=ROW)
            off = 0
            gq = 0
            for s in subs_last if w == wpc - 1 else subs:
                nc.gpsimd.dma_gather(
                    G3w[:, off // P : (off + s) // P, :],
                    ft[w * wrows : (w + 1) * wrows, :],
                    idx_t[
                        :,
                        (w * c_cap + off) // 16 : (w * c_cap + off + s) // 16,
                    ],
                    num_idxs=s,
                    num_idxs_reg=s,
                    elem_size=ROW,
                    queue_num=gq % 4,
                )
                gq += 1
                off += s
            G3 = G[:].rearrange("p (c e) -> p c e", e=ROW)

            def piece(lo, hi, w=w, G3=G3):
                n = hi - lo
                cum = wk_pool.tile([P, (cc + 1) * DIM], f32, tag="cum")
                nc.vector.memset(cum[:, DIM - 1 : DIM], 0)
                nc.vector._custom_dve(
                    cumsqdiff,
                    out=cum[:, DIM : (n + 1) * DIM],
                    in0=G3[:, lo:hi, 0:DIM],
                    in1=kr3_t[w][:, lo:hi, :],
                )
                cum3 = cum[:].rearrange("p (c d) -> p c d", d=DIM)
                dpd = wk_pool.tile([P, cc], f32, tag="dpd")
                nc.vector.scalar_tensor_tensor(
                    out=dpd[:, 0:n].unsqueeze(-1),
                    in0=cum3[:, 1 : n + 1, DIM - 1 : DIM],
                    scalar=DELTA,
                    in1=cum3[:, 0:n, DIM - 1 : DIM],
                    op0=mybir.AluOpType.add,
                    op1=mybir.AluOpType.subtract,
                )
                wslice = w23[:, w * cc + lo : w * cc + hi, 0:1]
                nc.vector.reciprocal(wslice, dpd[:, 0:n].unsqueeze(-1))
                vv = G3[:, lo:hi, DIM : DIM + 2].bitcast(f32)
                nc.vector.tensor_tensor(
                    out=w23[:, w * cc + lo : w * cc + hi, 1:2],
                    in0=wslice,
                    in1=vv,
                    op=mybir.AluOpType.mult,
                )
                nc.scalar.dma_start(
                    out=w2o[:, (w * cc + lo) * 2 : (w * cc + hi) * 2],
                    in_=w2[:, (w * cc + lo) * 2 : (w * cc + hi) * 2],
                )

            if w == wpc - 1:
                # per-sub pieces so the final compute tail is one 512-chunk
                o = 0
                for s in subs_last:
                    piece(o // P, (o + s) // P)
                    o += s
            else:
                piece(0, cc)
            if w + 2 < wpc:
                load_kr(w + 2)  # after this window's out-DMA in scalar order

    nc.compile()
    _BUILD_CACHE[key] = nc
    return nc


def _bf16(x):
    import ml_dtypes

    return np.asarray(x, dtype=np.float32).astype(ml_dtypes.bfloat16)


def _make_fused_table(keys_table, values_table, cap):
    import ml_dtypes

    fused = np.zeros((cap, ROW), dtype=ml_dtypes.bfloat16)
    fused[:, :DIM] = _bf16(keys_table)
    v32 = np.ascontiguousarray(np.asarray(values_table, dtype=np.float32).reshape(-1))
    fused_u16 = fused.view(np.uint16)
    fused_u16[:, DIM : DIM + 2] = v32.view(np.uint16).reshape(cap, 2)
    return fused


def _host_prep(key, indices, keys_table, values_table, wrows=WROWS, wpc=WPC,
               n_cores=N_CORES):
    """Returns (fused, per-core in_maps, epilogue info, c_cap)."""
    cap = keys_table.shape[0]
    batch, k = indices.shape
    n_windows = n_cores * wpc
    fused = _make_fused_table(keys_table, values_table, cap)
    key_bf = _bf16(key)  # [batch, DIM]

    i_all = np.asarray(indices).reshape(-1).astype(np.int64)  # [batch*k]
    b_all = np.repeat(np.arange(batch, dtype=np.int64), k)
    w_id = i_all // wrows
    local = (i_all - w_id * wrows).astype(np.int16)

    # sort by (window, local row): ascending DRAM addresses within each
    # window give the gather's DMA descriptors page/bank locality
    order = np.lexsort((local, w_id))
    w_sorted = w_id[order]
    local_sorted = local[order]
    b_sorted = b_all[order]

    counts = np.bincount(w_id, minlength=n_windows)
    c_cap = max(P, int(-(-counts.max() // P)) * P)

    # Engine-contiguous slot permutation: the gather ucode routes the
    # descriptor for slot position 128*k + first[l] + offs[g] to DMA engine
    # l, which processes its descriptors serially.  Placing CONSECUTIVE
    # sorted rows on one engine's stream turns its serial random reads into
    # an ascending walk (DRAM page/row-buffer locality).
    _first = np.array(
        [0, 64, 4, 68, 8, 72, 12, 76, 16, 80, 20, 84, 24, 88, 28, 92]
    )
    _offs = np.array([0, 1, 2, 3, 32, 33, 34, 35])

    def _eperm(s):
        pp = np.empty(s, dtype=np.int64)
        spe = s // 16
        for el in range(16):
            for k in range(s // 128):
                for g in range(8):
                    pp[128 * k + _first[el] + _offs[g]] = el * spe + k * 8 + g
        return pp

    def _window_perm(subs):
        pp = np.empty(c_cap, dtype=np.int64)
        off = 0
        for s in subs:
            pp[off : off + s] = off + _eperm(s)
            off += s
        return pp

    subs_main = []
    r = c_cap
    while r > 0:
        t = min(r, 1024)
        subs_main.append(t)
        r -= t
    # must mirror the device's subs_last construction exactly
    subs_last = []
    r = c_cap
    while r > 0:
        t = min(r, 512)
        subs_last.append(t)
        r -= t
    pos_to_pair = _window_perm(subs_main)
    pos_to_pair_last = _window_perm(subs_last)

    # [n_windows, c_cap] slot arrays, dup-padded with row 0 (all-valid lists:
    # the -1-skip path is flaky on HW; pads are gathered and dropped on host)
    local_pad = np.zeros((n_windows, c_cap), dtype=np.int16)
    b_pad = np.zeros((n_windows, c_cap), dtype=np.int64)
    valid = np.zeros((n_windows, c_cap), dtype=bool)
    starts = np.concatenate([[0], np.cumsum(counts)])
    for w in range(n_windows):
        n = counts[w]
        sl = slice(starts[w], starts[w] + n)
        loc_seq = np.zeros(c_cap, dtype=np.int16)
        loc_seq[:n] = local_sorted[sl]
        b_seq = np.zeros(c_cap, dtype=np.int64)
        b_seq[:n] = b_sorted[sl]
        v_seq = np.zeros(c_cap, dtype=bool)
        v_seq[:n] = True
        pp = pos_to_pair_last if (w % wpc) == wpc - 1 else pos_to_pair
        local_pad[w] = loc_seq[pp]
        b_pad[w] = b_seq[pp]
        valid[w] = v_seq[pp]

    cc = c_cap // P
    in_maps = []
    for c in range(n_cores):
        ws = slice(c * wpc, (c + 1) * wpc)
        # int16 idx tile: index j of window w at [16*rep + j%16, w*c_cap//16 + j//16]
        lp = local_pad[ws]  # [wpc, c_cap]
        arr = lp.reshape(wpc, c_cap // 16, 16).transpose(2, 0, 1).reshape(
            16, wpc * (c_cap // 16)
        )
        idx_tile = np.tile(arr, (8, 1))  # replicate to 128 partitions
        # kr: slot j of window w lands at [j%128, w*cc + j//128]
        bp = b_pad[ws].reshape(wpc, cc, P)  # [w, col, p]
        kr_arr = key_bf[bp]  # [w, col, p, DIM]
        kr_tile = np.ascontiguousarray(
            kr_arr.transpose(2, 0, 1, 3).reshape(P, wpc * cc * DIM)
        )
        in_maps.append(
            {
                "ft_shard": fused[c * wpc * wrows : (c + 1) * wpc * wrows],
                "kr": kr_tile,
                "idxs": np.ascontiguousarray(idx_tile),
            }
        )
    epi = (b_pad, valid, batch)
    return in_maps, epi, c_cap


def _epilogue(results, epi, wpc=WPC, n_cores=N_CORES):
    b_pad, valid, batch = epi
    den = np.zeros(batch, dtype=np.float64)
    num = np.zeros(batch, dtype=np.float64)
    c_cap = b_pad.shape[1]
    cc = c_cap // P
    for c in range(n_cores):
        w2 = results[c]["w2"].reshape(P, wpc * cc, 2)  # [p, col, 2]
        # slot j of window w is at [j%128, w*cc + j//128]
        w2_slots = w2.reshape(P, wpc, cc, 2).transpose(1, 2, 0, 3).reshape(
            wpc, c_cap, 2
        )
        v = valid[c * wpc : (c + 1) * wpc]
        b = b_pad[c * wpc : (c + 1) * wpc]
        den += np.bincount(b[v], weights=w2_slots[..., 0][v], minlength=batch)
        num += np.bincount(b[v], weights=w2_slots[..., 1][v], minlength=batch)
    return (num / den).astype(np.float32)


LAST_RESULTS = None


def kernel(key, indices, keys_table, values_table):
    global LAST_RESULTS
    from concourse.bass_utils import run_bass_kernel_spmd

    in_maps, epi, c_cap = _host_prep(key, indices, keys_table, values_table)
    nc = _build(c_cap)
    res = run_bass_kernel_spmd(nc, in_maps, core_ids=list(range(N_CORES)))
    LAST_RESULTS = res
    return _epilogue(res.results, epi)



# revision 41
# speedup vs baseline: 1.0437x; 1.0290x over previous
"""DND kNN kernel v4 — dma_gather over 4 SWDGE queues.

Sharding: the fused table (bf16 keys + f32-value bits, padded to 256 B rows)
is split row-wise into 8 shards of 250 000 rows, one per core; each shard is
further split into 8 windows of 31 250 rows so local row ids fit int16 (the
dma_gather index dtype).  Each (query, neighbor) pair is routed on the host to
the core owning its table row, sorted by (window, local row), and padded per
window to a static slot count C_cap (multiple of 128).  Host-side prep also
builds, per core, the int16 index tiles (wrapped in 16 partitions, replicated
for the 8 Q7 cores) and a slot-aligned copy of the query vectors (kr).

Device, per core: each window's rows are fetched by 1024-idx dma_gather calls
rotated across all 4 SWDGE queues (num_swdge_queues=4).  A single gather's
drain is latency-bound — its descriptors stripe over 16 DMA engines that each
process ~64 serial ~140 ns random 256 B HBM reads — so queue rotation is the
big lever: 4 in-flight ring drains ≈ 2.8x over one queue (290 us -> 103 us).
Slots are additionally permuted so each DMA engine's serial descriptor stream
walks ascending table rows, and the last window is issued as 512-idx chunks
so the end-of-kernel drains overlap across queues.  A custom DVE op computes
a running sum of (row - query)^2 per partition; per-row squared distances
come out as differences of the cumsum at 64-element boundaries; w =
1/(d2+delta) and w*v land in a [128, S, 2] tile that is DMA'd out.  Host
epilogue: segmented sum of (w, w*v) by query id and the final division — the
unshard step for this table-parallel sharding.

Remaining time budget (measured): ~17 us prologue gated by the Q7 'mlp'
library image DMA (fixed), ~71 us of gather drain at the 64-outstanding
latency ceiling, ~12 us drain tail + end barrier.
"""

import sys

import numpy as np

for _p in ("/opt/trn_rl_repo",):
    if _p not in sys.path:
        sys.path.insert(0, _p)

CAPACITY = 2_000_000
DIM = 64
K = 50
BATCH = 4096
DELTA = 1e-3
N_CORES = 8
P = 128
ROW = 128  # bf16 elems per fused row = 256 B
WPC = 8  # windows per core
N_WINDOWS = N_CORES * WPC  # 64
WROWS = CAPACITY // N_WINDOWS  # 31250 rows per window, int16-addressable

_BUILD_CACHE = {}


def _register_cumsqdiff():
    from concourse import dve_ops
    from concourse.dve_spec import AluOp, Spec, Src0, Src1, _has_src1, lower, scan, sq
    from concourse.dve_uop import DveOpSpec

    for op in dve_ops.OPS:
        if op.name == "CUMSQDIFF_ANT":
            return op

    def _ref(in0, in1, c0, c1, c2):
        a = in0.astype(np.float32).reshape(in0.shape[0], -1)
        b = in1.astype(np.float32).reshape(in1.shape[0], -1)
        d = a - b
        s = np.cumsum(d * d, axis=1, dtype=np.float32)
        return s.reshape(in0.shape)

    spec = Spec(body=scan(AluOp.ADD, sq(Src0 - Src1)), reference=_ref)
    name = "CUMSQDIFF_ANT"
    opcode = dve_ops._CUSTOM_DVE_ROW_BASE + len(dve_ops.OPS)
    shas = {}
    for ver in ("v3", "v4"):
        s = DveOpSpec(
            name=name, opcode=opcode, uops=lower(spec, ver=ver), rd1_en=_has_src1(spec)
        )
        shas[ver] = s.sha(ver)
    op = dve_ops.DveOp(name, spec, subdim=False, uops_sha=shas)
    dve_ops.OPS.append(op)
    dve_ops.CUSTOM_DVE_SPECS[name] = spec
    dve_ops._SUB_OPCODE_FOR_NAME[name] = opcode
    return op


def _build(c_cap, wrows=WROWS, wpc=WPC, num_devices=N_CORES):
    """Per-core program. c_cap = slots per window (multiple of 128)."""
    key = (c_cap, wrows, wpc, num_devices)
    if key in _BUILD_CACHE:
        return _BUILD_CACHE[key]

    from contextlib import ExitStack

    import concourse.bacc as bacc
    import concourse.tile as tile
    from concourse import mybir

    f32 = mybir.dt.float32
    bf16 = mybir.dt.bfloat16
    i16 = mybir.dt.int16
    cumsqdiff = _register_cumsqdiff()

    cc = c_cap // P  # gather out columns per window
    S = wpc * cc  # total slot columns per core

    nc = bacc.Bacc(
        "TRN2",
        target_bir_lowering=False,
        debug=False,
        num_devices=num_devices,
        num_swdge_queues=4,
    )
    SUBMAX = 1024  # dma_gather num_idxs hard cap (measured; >1024 faults)
    subs = []
    r = c_cap
    while r > 0:
        s = min(r, SUBMAX)
        subs.append(s)
        r -= s
    ft = nc.dram_tensor("ft_shard", [wpc * wrows, ROW], bf16, kind="ExternalInput")
    kr = nc.dram_tensor("kr", [P, S * DIM], bf16, kind="ExternalInput")
    idxs = nc.dram_tensor("idxs", [P, wpc * c_cap // 16], i16, kind="ExternalInput")
    w2o = nc.dram_tensor("w2", [P, S * 2], f32, kind="ExternalOutput")

    with tile.TileContext(nc) as tc, ExitStack() as ctx:
        io_pool = ctx.enter_context(tc.tile_pool(name="io", bufs=1))
        g_pool = ctx.enter_context(tc.tile_pool(name="g", bufs=2))
        wk_pool = ctx.enter_context(tc.tile_pool(name="wk", bufs=2))

        # chunked loads: smaller DMAs interleave better on the engine queues
        # than one monolithic transfer (measured)
        idx_t = io_pool.tile([P, wpc * c_cap // 16], i16, tag="idx")
        for w in range(wpc):
            eng = nc.sync if w % 2 == 0 else nc.scalar
            eng.dma_start(
                out=idx_t[:, w * (c_cap // 16) : (w + 1) * (c_cap // 16)],
                in_=idxs[:, w * (c_cap // 16) : (w + 1) * (c_cap // 16)],
            )
        w2 = io_pool.tile([P, S * 2], f32, tag="w2")
        w23 = w2[:].rearrange("p (s two) -> p s two", two=2)

        kr_all = io_pool.tile([P, S * DIM], bf16, tag="kr_all")
        for w in range(wpc):
            eng = nc.scalar if w % 2 == 0 else nc.sync
            eng.dma_start(
                out=kr_all[:, w * cc * DIM : (w + 1) * cc * DIM],
                in_=kr[:, w * cc * DIM : (w + 1) * cc * DIM],
            )
        kr3 = kr_all[:].rearrange("p (s d) -> p s d", d=DIM)

        # last window gathers in 512-chunks: spreads the final drains over
        # all 4 queues so the end-of-kernel DRAIN tail is shorter
        subs_last = []
        r = c_cap
        while r > 0:
            t = min(r, 512)
            subs_last.append(t)
            r -= t

        gq = 0  # rotate SWDGE queues so DMA ring drains overlap
        for w in range(wpc):
            G = g_pool.tile([P, cc * ROW], bf16, tag="G")
            G3w = G[:].rearrange("p (c e) -> p c e", e=ROW)
            off = 0
            gq = 0
            for s in subs_last if w == wpc - 1 else subs:
                nc.gpsimd.dma_gather(
                    G3w[:, off // P : (off + s) // P, :],
                    ft[w * wrows : (w + 1) * wrows, :],
                    idx_t[
                        :,
                        (w * c_cap + off) // 16 : (w * c_cap + off + s) // 16,
                    ],
                    num_idxs=s,
                    num_idxs_reg=s,
                    elem_size=ROW,
                    queue_num=gq % 4,
                )
                gq += 1
                off += s
            G3 = G[:].rearrange("p (c e) -> p c e", e=ROW)

            def piece(lo, hi, w=w, G3=G3):
                n = hi - lo
                cum = wk_pool.tile([P, (cc + 1) * DIM], f32, tag="cum")
                nc.vector.memset(cum[:, DIM - 1 : DIM], 0)
                nc.vector._custom_dve(
                    cumsqdiff,
                    out=cum[:, DIM : (n + 1) * DIM],
                    in0=G3[:, lo:hi, 0:DIM],
                    in1=kr3[:, w * cc + lo : w * cc + hi, :],
                )
                cum3 = cum[:].rearrange("p (c d) -> p c d", d=DIM)
                dpd = wk_pool.tile([P, cc], f32, tag="dpd")
                nc.vector.scalar_tensor_tensor(
                    out=dpd[:, 0:n].unsqueeze(-1),
                    in0=cum3[:, 1 : n + 1, DIM - 1 : DIM],
                    scalar=DELTA,
                    in1=cum3[:, 0:n, DIM - 1 : DIM],
                    op0=mybir.AluOpType.add,
                    op1=mybir.AluOpType.subtract,
                )
                wslice = w23[:, w * cc + lo : w * cc + hi, 0:1]
                nc.vector.reciprocal(wslice, dpd[:, 0:n].unsqueeze(-1))
                vv = G3[:, lo:hi, DIM : DIM + 2].bitcast(f32)
                nc.vector.tensor_tensor(
                    out=w23[:, w * cc + lo : w * cc + hi, 1:2],
                    in0=wslice,
                    in1=vv,
                    op=mybir.AluOpType.mult,
                )
                nc.scalar.dma_start(
                    out=w2o[:, (w * cc + lo) * 2 : (w * cc + hi) * 2],
                    in_=w2[:, (w * cc + lo) * 2 : (w * cc + hi) * 2],
                )

            if w == wpc - 1:
                # per-sub pieces so the final compute tail is one 512-chunk
                o = 0
                for s in subs_last:
                    piece(o // P, (o + s) // P)
                    o += s
            else:
                piece(0, cc)

    nc.compile()
    _BUILD_CACHE[key] = nc
    return nc


def _bf16(x):
    import ml_dtypes

    return np.asarray(x, dtype=np.float32).astype(ml_dtypes.bfloat16)


def _make_fused_table(keys_table, values_table, cap):
    import ml_dtypes

    fused = np.zeros((cap, ROW), dtype=ml_dtypes.bfloat16)
    fused[:, :DIM] = _bf16(keys_table)
    v32 = np.ascontiguousarray(np.asarray(values_table, dtype=np.float32).reshape(-1))
    fused_u16 = fused.view(np.uint16)
    fused_u16[:, DIM : DIM + 2] = v32.view(np.uint16).reshape(cap, 2)
    return fused


def _host_prep(key, indices, keys_table, values_table, wrows=WROWS, wpc=WPC,
               n_cores=N_CORES):
    """Returns (fused, per-core in_maps, epilogue info, c_cap)."""
    cap = keys_table.shape[0]
    batch, k = indices.shape
    n_windows = n_cores * wpc
    fused = _make_fused_table(keys_table, values_table, cap)
    key_bf = _bf16(key)  # [batch, DIM]

    i_all = np.asarray(indices).reshape(-1).astype(np.int64)  # [batch*k]
    b_all = np.repeat(np.arange(batch, dtype=np.int64), k)
    w_id = i_all // wrows
    local = (i_all - w_id * wrows).astype(np.int16)

    # sort by (window, local row): ascending DRAM addresses within each
    # window give the gather's DMA descriptors page/bank locality
    order = np.lexsort((local, w_id))
    w_sorted = w_id[order]
    local_sorted = local[order]
    b_sorted = b_all[order]

    counts = np.bincount(w_id, minlength=n_windows)
    c_cap = max(P, int(-(-counts.max() // P)) * P)

    # Engine-contiguous slot permutation: the gather ucode routes the
    # descriptor for slot position 128*k + first[l] + offs[g] to DMA engine
    # l, which processes its descriptors serially.  Placing CONSECUTIVE
    # sorted rows on one engine's stream turns its serial random reads into
    # an ascending walk (DRAM page/row-buffer locality).
    _first = np.array(
        [0, 64, 4, 68, 8, 72, 12, 76, 16, 80, 20, 84, 24, 88, 28, 92]
    )
    _offs = np.array([0, 1, 2, 3, 32, 33, 34, 35])

    def _eperm(s):
        pp = np.empty(s, dtype=np.int64)
        spe = s // 16
        for el in range(16):
            for k in range(s // 128):
                for g in range(8):
                    pp[128 * k + _first[el] + _offs[g]] = el * spe + k * 8 + g
        return pp

    def _window_perm(subs):
        pp = np.empty(c_cap, dtype=np.int64)
        off = 0
        for s in subs:
            pp[off : off + s] = off + _eperm(s)
            off += s
        return pp

    subs_main = []
    r = c_cap
    while r > 0:
        t = min(r, 1024)
        subs_main.append(t)
        r -= t
    # must mirror the device's subs_last construction exactly
    subs_last = []
    r = c_cap
    while r > 0:
        t = min(r, 512)
        subs_last.append(t)
        r -= t
    pos_to_pair = _window_perm(subs_main)
    pos_to_pair_last = _window_perm(subs_last)

    # [n_windows, c_cap] slot arrays, dup-padded with row 0 (all-valid lists:
    # the -1-skip path is flaky on HW; pads are gathered and dropped on host)
    local_pad = np.zeros((n_windows, c_cap), dtype=np.int16)
    b_pad = np.zeros((n_windows, c_cap), dtype=np.int64)
    valid = np.zeros((n_windows, c_cap), dtype=bool)
    starts = np.concatenate([[0], np.cumsum(counts)])
    for w in range(n_windows):
        n = counts[w]
        sl = slice(starts[w], starts[w] + n)
        loc_seq = np.zeros(c_cap, dtype=np.int16)
        loc_seq[:n] = local_sorted[sl]
        b_seq = np.zeros(c_cap, dtype=np.int64)
        b_seq[:n] = b_sorted[sl]
        v_seq = np.zeros(c_cap, dtype=bool)
        v_seq[:n] = True
        pp = pos_to_pair_last if (w % wpc) == wpc - 1 else pos_to_pair
        local_pad[w] = loc_seq[pp]
        b_pad[w] = b_seq[pp]
        valid[w] = v_seq[pp]

    cc = c_cap // P
    in_maps = []
    for c in range(n_cores):
        ws = slice(c * wpc, (c + 1) * wpc)
        # int16 idx tile: index j of window w at [16*rep + j%16, w*c_cap//16 + j//16]
        lp = local_pad[ws]  # [wpc, c_cap]
        arr = lp.reshape(wpc, c_cap // 16, 16).transpose(2, 0, 1).reshape(
            16, wpc * (c_cap // 16)
        )
        idx_tile = np.tile(arr, (8, 1))  # replicate to 128 partitions
        # kr: slot j of window w lands at [j%128, w*cc + j//128]
        bp = b_pad[ws].reshape(wpc, cc, P)  # [w, col, p]
        kr_arr = key_bf[bp]  # [w, col, p, DIM]
        kr_tile = np.ascontiguousarray(
            kr_arr.transpose(2, 0, 1, 3).reshape(P, wpc * cc * DIM)
        )
        in_maps.append(
            {
                "ft_shard": fused[c * wpc * wrows : (c + 1) * wpc * wrows],
                "kr": kr_tile,
                "idxs": np.ascontiguousarray(idx_tile),
            }
        )
    epi = (b_pad, valid, batch)
    return in_maps, epi, c_cap


def _epilogue(results, epi, wpc=WPC, n_cores=N_CORES):
    b_pad, valid, batch = epi
    den = np.zeros(batch, dtype=np.float64)
    num = np.zeros(batch, dtype=np.float64)
    c_cap = b_pad.shape[1]
    cc = c_cap // P
    for c in range(n_cores):
        w2 = results[c]["w2"].reshape(P, wpc * cc, 2)  # [p, col, 2]
        # slot j of window w is at [j%128, w*cc + j//128]
        w2_slots = w2.reshape(P, wpc, cc, 2).transpose(1, 2, 0, 3).reshape(
            wpc, c_cap, 2
        )
        v = valid[c * wpc : (c + 1) * wpc]
        b = b_pad[c * wpc : (c + 1) * wpc]
        den += np.bincount(b[v], weights=w2_slots[..., 0][v], minlength=batch)
        num += np.bincount(b[v], weights=w2_slots[..., 1][v], minlength=batch)
    return (num / den).astype(np.float32)


LAST_RESULTS = None


def kernel(key, indices, keys_table, values_table):
    global LAST_RESULTS
    from concourse.bass_utils import run_bass_kernel_spmd

    in_maps, epi, c_cap = _host_prep(key, indices, keys_table, values_table)
    nc = _build(c_cap)
    res = run_bass_kernel_spmd(nc, in_maps, core_ids=list(range(N_CORES)))
    LAST_RESULTS = res
    return _epilogue(res.results, epi)

